# revision 16
# baseline (speedup 1.0000x reference)
"""Trainium2 Bass kernel for a causal self-attention block (GQA + gated value
embedding + RoPE + QK-RMSNorm), sharded over 8 NeuronCores.

Sharding: 8 cores = 2 (batch) x 4 (kv-head groups).  Each core computes, for
its batch b and head-group g (4 q-heads + 1 kv-head):
    q/k/v projections, gated ve addition, RoPE, RMSNorm, causal attention,
    and the partial output projection  y_g @ Wproj[g*512:(g+1)*512, :].
The host sums the 4 per-group partials for each batch (the Wproj
contraction distributes over head groups).

All matmul operands are bf16 (PSUM accumulation stays fp32); the host
pre-transposes x so no on-device x transposes are needed; attention
S / AV / softmax-denominator run in one merged pipeline with the output
projection interleaved per chunk so the PE stream stays dense.

Self-contained: hardcodes shapes; accepts FULL inputs, returns FULL output.
"""

from contextlib import ExitStack

import ml_dtypes
import numpy as np

import concourse.bacc as bacc
import concourse.bass as bass
import concourse.mybir as mybir
import concourse.tile as tile
from concourse.bass_utils import run_bass_kernel_spmd
from concourse.masks import make_identity

F32 = mybir.dt.float32
BF16 = mybir.dt.bfloat16
I32 = mybir.dt.int32
AF = mybir.ActivationFunctionType
ALU = mybir.AluOpType
AX = mybir.AxisListType
BF = ml_dtypes.bfloat16

B, C, HD, NHL, GC = 2, 2048, 128, 4, 32  # NHL = local q heads per core
T = 2048
TB = T // 128   # token blocks
CT = C // 128   # contraction tiles for qkv
NCH = T // 512  # i-chunks for attention
OC = C // 512   # output chunks for proj
ISQ = 1.0 / float(np.sqrt(128.0))
S128 = float(np.sqrt(128.0))
QKS = 64.0  # host pre-scale on Wq/Wk so fp8 weights sit in the normal range
RSQRT_MAGIC = 0x5F3759DF
F8 = mybir.dt.float8e4
F8NP = ml_dtypes.float8_e4m3
DR = mybir.MatmulPerfMode.DoubleRow


def _bcast(ap_, idx, count):
    """Insert a step-0 (broadcast) dim at position idx of the AP dims."""
    lst = [list(p) for p in ap_.ap]
    lst.insert(idx, [0, count])
    return bass.AP(ap_.tensor, ap_.offset, lst)


def build():
    nc = bacc.Bacc("TRN2", target_bir_lowering=False, debug=False)
    xt = nc.dram_tensor("xt", [128, TB, CT, 128], BF16, kind="ExternalInput")
    wq = nc.dram_tensor("wq", [128, CT, 512], BF16, kind="ExternalInput")
    wkv = nc.dram_tensor("wkv", [128, CT, 2 * HD + 1], BF16, kind="ExternalInput")
    wproj = nc.dram_tensor("wproj", [128, NHL, OC, 512], BF16, kind="ExternalInput")
    ve = nc.dram_tensor("ve", [128, TB, HD], BF16, kind="ExternalInput")
    cosn = nc.dram_tensor("cosn", [128, TB, 64], BF16, kind="ExternalInput")
    sinn = nc.dram_tensor("sinn", [128, TB, 64], BF16, kind="ExternalInput")
    out = nc.dram_tensor("out", [T, C], F32, kind="ExternalOutput")

    with ExitStack() as stk:
        tc = stk.enter_context(tile.TileContext(nc))
        gpool = stk.enter_context(tc.tile_pool(name="gconst", bufs=1))
        warmsrc = gpool.tile([128, 128], BF16)
        nc.vector.memset(warmsrc, 1.0)
        ident = gpool.tile([128, 128], F32)
        make_identity(nc, ident)
        identb = gpool.tile([128, 128], BF16)
        nc.vector.tensor_copy(out=identb, in_=ident)
        ones_f = gpool.tile([128, 128], F32)
        nc.vector.memset(ones_f, 1.0)
        onesb = gpool.tile([128, 128], BF16)
        nc.vector.tensor_copy(out=onesb, in_=ones_f)
        # preload the Exp activation table while DMAs land
        tl_i = gpool.tile([128, 1], F32)
        nc.vector.memset(tl_i, 0.0)
        tl_o = gpool.tile([128, 1], F32)
        nc.scalar.activation(tl_o, tl_i, AF.Exp, scale=-1.0)

        persist = stk.enter_context(tc.tile_pool(name="persist", bufs=1))
        qkT = persist.tile([128, NHL + 1, T], BF16)  # [d, h, t]; h=4 is k
        vS = persist.tile([128, TB, HD], BF16)       # [t%128, t//128, d]
        yT = persist.tile([128, NHL, T], BF16)       # [d, h, t]

        wA = stk.enter_context(tc.tile_pool(name="wA", bufs=1))
        wq_sb = wA.tile([128, CT, 512], BF16)
        wkv_sb = wA.tile([128, CT, 2 * HD + 1], BF16)
        wp_sb = wA.tile([128, NHL, OC, 512], BF16)
        ve_sb = wA.tile([128, TB, HD], BF16)
        cos_sb = wA.tile([128, TB, 64], BF16)
        sin_sb = wA.tile([128, TB, 64], BF16)

        # DMA kickoff, interleaved across the two DGE queues so the first
        # qkv block's operands all land within a few us.
        xns = {}
        xpool = stk.enter_context(tc.tile_pool(name="xpool", bufs=5))
        for tb in range(4):
            xns[tb] = xpool.tile([128, CT, 128], BF16, tag="xn", name="xn")
        nc.gpsimd.dma_start(out=xns[0], in_=xt[:, 0])
        nc.sync.dma_start(out=wq_sb[:, 0:4, :], in_=wq[:, 0:4, :])
        nc.gpsimd.dma_start(out=wq_sb[:, 4:8, :], in_=wq[:, 4:8, :])
        nc.sync.dma_start(out=wq_sb[:, 8:12, :], in_=wq[:, 8:12, :])
        nc.gpsimd.dma_start(out=wq_sb[:, 12:16, :], in_=wq[:, 12:16, :])
        nc.sync.dma_start(out=wkv_sb[:, 0:8, :], in_=wkv[:, 0:8, :])
        nc.gpsimd.dma_start(out=wkv_sb[:, 8:16, :], in_=wkv[:, 8:16, :])
        nc.sync.dma_start(out=xns[1], in_=xt[:, 1])
        nc.gpsimd.dma_start(out=xns[2], in_=xt[:, 2])
        nc.sync.dma_start(out=xns[3], in_=xt[:, 3])
        nc.gpsimd.dma_start(out=cos_sb, in_=cosn[:, :, :])
        nc.sync.dma_start(out=ve_sb, in_=ve[:, :, :])
        nc.gpsimd.dma_start(out=sin_sb, in_=sinn[:, :, :])
        nc.gpsimd.dma_start(out=wp_sb, in_=wproj[:, :, :, :])

        # PE warmup so HAM ramps toward full clock while the DMAs land.
        with tc.tile_pool(name="warm", bufs=2, space="PSUM") as warm:
            for _ in range(40):
                w_ps = warm.tile([128, 128], BF16, tag="wps", name="wps")
                nc.tensor.transpose(w_ps, warmsrc, warmsrc)

        # staged PSUM scopes: qkv-accumulation pools close before the
        # attention pools open; the transpose pool stays open into the
        # attention phase so the last block's transposes can be emitted
        # after the first attention items (PE never waits on the tail
        # of the phase-A DVE chain).
        psA1 = ExitStack()
        psq = psA1.enter_context(tc.tile_pool(name="psq", bufs=3, space="PSUM"))
        pskv = psA1.enter_context(tc.tile_pool(name="pskv", bufs=3, space="PSUM"))
        psA2 = ExitStack()
        pst = psA2.enter_context(
            tc.tile_pool(name="pst", bufs=2, space="PSUM", side="right"))

        sbA = stk.enter_context(tc.tile_pool(name="sbA", bufs=2))
        qkh = stk.enter_context(tc.tile_pool(name="qkh", bufs=3))

        def emit_transposes(pend):
            pqk, pt0 = pend
            for hh in range(NHL + 1):
                tq_ps = pst.tile([128, 128], BF16, tag="tps")
                nc.tensor.transpose(
                    tq_ps, pqk[:, hh * 128:(hh + 1) * 128], identb)
                nc.scalar.copy(out=qkT[:, hh, pt0:pt0 + 128], in_=tq_ps)

        # ---------------- phase A: qkv + rope + rmsnorm + transposes --------
        with nc.named_scope("phaseA"):
            pends = []  # (qkhat, t0) awaiting transpose into qkT, lag 2
            for tb in range(TB):
                t0 = tb * 128
                xn = xns.pop(tb)
                q_ps = psq.tile([128, NHL * HD], F32, tag="qps")
                kv_ps = pskv.tile([128, 2 * HD + 1], F32, tag="kvps")

                # qkv matmuls (k, v and the gate column fused in one rhs);
                # transposes of the block-before-last are interleaved
                # (2-block lag gives the DVE chain time to finish, so the
                # PE never waits on qkhat)
                pend = pends.pop(0) if len(pends) == 2 else None
                for ct in range(CT):
                    nc.tensor.matmul(
                        q_ps, lhsT=xn[:, ct, :], rhs=wq_sb[:, ct, :],
                        start=(ct == 0), stop=(ct == CT - 1))
                    nc.tensor.matmul(
                        kv_ps, lhsT=xn[:, ct, :], rhs=wkv_sb[:, ct, :],
                        start=(ct == 0), stop=(ct == CT - 1))
                    if pend is not None and ct in (2, 5, 8, 11, 14):
                        hh = (ct - 2) // 3
                        pqk, pt0 = pend
                        tq_ps = pst.tile([128, 128], BF16, tag="tps")
                        nc.tensor.transpose(
                            tq_ps, pqk[:, hh * 128:(hh + 1) * 128], identb)
                        nc.scalar.copy(
                            out=qkT[:, hh, pt0:pt0 + 128], in_=tq_ps)

                # prefetch: emitted after this block's matmuls so the ring
                # slot's previous reads are already ordered before the write
                if tb + 4 < TB:
                    xpf = xpool.tile([128, CT, 128], BF16, tag="xn", name="xn")
                    nc.sync.dma_start(out=xpf, in_=xt[:, tb + 4])
                    xns[tb + 4] = xpf

                # PSUM -> SBUF casts: q(4 heads)+k into one 5-head rope tile
                qkb = qkh.tile([128, (NHL + 1) * HD], BF16, tag="qkb")
                nc.scalar.copy(out=qkb[:, 0:512], in_=q_ps)
                nc.scalar.copy(out=qkb[:, 512:640], in_=kv_ps[:, 0:HD])
                vb = sbA.tile([128, HD], BF16, tag="vb")
                nc.scalar.copy(out=vb, in_=kv_ps[:, HD:2 * HD])

                # gate = sigmoid(z); z rides the kv matmul as weight col 256
                e_sb = sbA.tile([128, 1], F32, tag="esb")
                nc.scalar.activation(
                    e_sb, kv_ps[:, 2 * HD:2 * HD + 1], AF.Exp, scale=-1.0)
                nc.vector.tensor_scalar_add(e_sb, e_sb, 1.0)
                g_sb = sbA.tile([128, 1], F32, tag="gsb")
                nc.vector.reciprocal(g_sb, e_sb)
                # v = v_mm + sigmoid(z) * (2*ve)
                nc.vector.scalar_tensor_tensor(
                    out=vS[:, tb, :], in0=ve_sb[:, tb, :], scalar=g_sb,
                    in1=vb, op0=ALU.mult, op1=ALU.add)

                # ---- RoPE on q (4 heads) and k batched as 5 heads ----
                NH5 = NHL + 1
                cosB = _bcast(cos_sb[:, tb, :], 1, NH5)
                sinB = _bcast(sin_sb[:, tb, :], 1, NH5)
                qv = qkb.rearrange("p (h d) -> p h d", h=NH5)
                rh = sbA.tile([128, NH5 * HD], BF16, tag="rh")
                rhv = rh.rearrange("p (h d) -> p h d", h=NH5)
                tmp = sbA.tile([128, NH5, 64], BF16, tag="tmp")
                nc.vector.tensor_tensor(
                    out=rhv[:, :, 0:64], in0=qv[:, :, 0:64], in1=cosB,
                    op=ALU.mult)
                nc.vector.tensor_tensor(
                    out=tmp, in0=qv[:, :, 64:128], in1=sinB, op=ALU.mult)
                nc.vector.tensor_tensor(
                    out=rhv[:, :, 0:64], in0=rhv[:, :, 0:64], in1=tmp,
                    op=ALU.add)
                nc.vector.tensor_tensor(
                    out=rhv[:, :, 64:128], in0=qv[:, :, 64:128], in1=cosB,
                    op=ALU.mult)
                nc.vector.tensor_tensor(
                    out=tmp, in0=qv[:, :, 0:64], in1=sinB, op=ALU.mult)
                nc.vector.tensor_tensor(
                    out=rhv[:, :, 64:128], in0=rhv[:, :, 64:128], in1=tmp,
                    op=ALU.subtract)

                # ---- RMSNorm scales for 5 heads in one [128, 5] batch ----
                # rq = sqrt(128)*rsqrt(sum(q^2)) = rsqrt(mean(q^2)); the
                # sqrt(128) is folded into the Newton-step constants.
                sq2 = sbA.tile([128, NH5 * HD], BF16, tag="sq2")
                nc.vector.tensor_tensor(out=sq2, in0=rh, in1=rh, op=ALU.mult)
                red = sbA.tile([128, NH5], F32, tag="red")
                nc.vector.tensor_reduce(
                    out=red, in_=sq2.rearrange("p (h d) -> p h d", h=NH5),
                    axis=AX.X, op=ALU.add)
                rq = sbA.tile([128, NH5], F32, tag="rq")
                rqi = rq.bitcast(I32)
                nc.vector.tensor_scalar(
                    out=rqi, in0=red.bitcast(I32), scalar1=1, scalar2=None,
                    op0=ALU.logical_shift_right)
                nc.vector.tensor_scalar(
                    out=rqi, in0=rqi, scalar1=-1, scalar2=RSQRT_MAGIC,
                    op0=ALU.mult, op1=ALU.add)
                nt = sbA.tile([128, NH5], F32, tag="nt")
                nc.vector.tensor_tensor(out=nt, in0=rq, in1=rq, op=ALU.mult)
                nc.vector.tensor_tensor(out=nt, in0=nt, in1=red, op=ALU.mult)
                nc.vector.tensor_scalar(
                    out=nt, in0=nt, scalar1=-0.5 * S128, scalar2=1.5 * S128,
                    op0=ALU.mult, op1=ALU.add)
                nc.vector.tensor_tensor(out=rq, in0=rq, in1=nt, op=ALU.mult)

                qkhat = qkh.tile([128, NH5 * HD], BF16, tag="qkhat")
                for h5 in range(NH5):
                    nc.vector.tensor_scalar_mul(
                        qkhat[:, h5 * HD:(h5 + 1) * HD],
                        rhv[:, h5, :], rq[:, h5:h5 + 1])
                pends.append((qkhat, t0))

        psA1.close()  # free qkv PSUM banks for the attention pools

        # ---------------- phase B+C: attention + output projection ---------
        # c-outer / head-inner, software-pipelined S/exp one group ahead of
        # AV; softmax denominator via per-group DVE fold + one matmul; the
        # output projection for chunk c-1's four token blocks is interleaved
        # between heads of chunk c so the PE stream stays dense end-to-end.
        ptB = stk.enter_context(tc.tile_pool(name="ptB", bufs=6))
        smB = stk.enter_context(tc.tile_pool(name="smB", bufs=2))
        osb = stk.enter_context(tc.tile_pool(name="osb", bufs=2))
        with nc.named_scope("phaseBC"):
            psS = stk.enter_context(tc.tile_pool(name="psS", bufs=3, space="PSUM"))
            psy = stk.enter_context(tc.tile_pool(name="psy", bufs=2, space="PSUM"))
            psd = stk.enter_context(tc.tile_pool(name="psd", bufs=1, space="PSUM"))

            def s_group(meta, g):
                hh, c, i0 = meta["hh"], meta["c"], meta["i0"]
                pt = ptB.tile([128, 1024], BF16, tag="pt", name="pt")
                for s in range(2):
                    jb = 2 * g + s
                    io = max(0, 128 * jb - 512 * c)  # first causally-live col
                    sps = psS.tile([128, 512], F32, tag="sps", name="sps")
                    nc.tensor.matmul(
                        sps[:, io:512],
                        lhsT=qkT[:, NHL, jb * 128:(jb + 1) * 128],
                        rhs=qkT[:, hh, i0 + io:i0 + 512],
                        start=True, stop=True)
                    nc.scalar.activation(
                        pt[:, s * 512 + io:(s + 1) * 512],
                        sps[:, io:512], AF.Exp, scale=ISQ)
                for s in range(2):
                    jb = 2 * g + s
                    if jb >= 4 * c:  # diagonal block: zero j > i
                        io = 128 * (jb - 4 * c)
                        nc.gpsimd.affine_select(
                            out=pt[:, s * 512 + io:(s + 1) * 512],
                            in_=pt[:, s * 512 + io:(s + 1) * 512],
                            pattern=[[1, 512 - io]], compare_op=ALU.is_ge,
                            fill=0.0, base=0, channel_multiplier=-1)
                meta["pts"][g] = pt

            def av_group(meta, g, is_last):
                pt = meta["pts"].pop(g)
                yps = meta["yps"]
                c = meta["c"]
                for s in range(2):
                    jb = 2 * g + s
                    io = max(0, 128 * jb - 512 * c)
                    if jb == meta["first_jb"]:
                        io = 0  # start matmul must cover the full chunk
                    nc.tensor.matmul(
                        yps[:, io:512], lhsT=vS[:, jb, :],
                        rhs=pt[:, s * 512 + io:(s + 1) * 512],
                        start=(jb == meta["first_jb"]),
                        stop=(jb == meta["last_jb"]))
                # fold the two key blocks of this group on DVE, then one
                # per-group denominator matmul (halves the PE denominator)
                dps = meta["dps"]
                io0 = max(0, 128 * (2 * g) - 512 * c)
                io1 = max(0, 128 * (2 * g + 1) - 512 * c)
                acc = smB.tile([128, 512], BF16, tag="acc", name="acc")
                nc.vector.tensor_tensor(
                    out=acc[:, io1:512], in0=pt[:, io1:512],
                    in1=pt[:, 512 + io1:1024], op=ALU.add)
                if io1 > io0:
                    nc.vector.tensor_copy(
                        out=acc[:, io0:io1], in_=pt[:, io0:io1])
                nc.tensor.matmul(
                    dps[:, io0:512], lhsT=onesb, rhs=acc[:, io0:512],
                    start=(g == meta["order0"]),
                    stop=(g == meta["order_last"]))
                if is_last:
                    hh, i0 = meta["hh"], meta["i0"]
                    rcp = smB.tile([128, 512], F32, tag="rcp", name="rcp")
                    nc.vector.reciprocal_approx_fast(out=rcp, in_=dps)
                    nc.vector.tensor_tensor(
                        out=yT[:, hh, i0:i0 + 512], in0=yps, in1=rcp,
                        op=ALU.mult)

            def attn_item(c, hh, prev):
                yps = psy.tile([128, 512], F32, tag="yps", name="yps")
                dps = psd.tile([128, 512], F32, tag="dps", name="dps")
                ngrp = (4 * c + 4) // 2
                order = list(range(ngrp - 2, ngrp)) + list(range(ngrp - 2))
                meta = dict(hh=hh, c=c, i0=c * 512, yps=yps, dps=dps,
                            pts={}, first_jb=2 * order[0],
                            last_jb=2 * order[-1] + 1,
                            order0=order[0], order_last=order[-1])
                for idx, g in enumerate(order):
                    s_group(meta, g)
                    if prev is not None:
                        av_group(*prev)
                    prev = (meta, g, idx == len(order) - 1)
                return prev

            # first attention items run while the last qkv blocks' DVE
            # chains finish; their transposes are emitted in between.
            prev = attn_item(0, 0, None)
            emit_transposes(pends[0])
            prev = attn_item(0, 1, prev)
            emit_transposes(pends[1])
            psA2.close()  # free the transpose PSUM banks
            psC = stk.enter_context(tc.tile_pool(name="psC", bufs=2, space="PSUM"))

            def proj_item(tb):
                t0 = tb * 128
                o_sb = osb.tile([128, C], F32, tag="osb", name="osb")
                for oc in range(OC):
                    o_ps = psC.tile([128, 512], F32, tag="ops", name="ops")
                    for hh in range(NHL):
                        nc.tensor.matmul(
                            o_ps, lhsT=yT[:, hh, t0:t0 + 128],
                            rhs=wp_sb[:, hh, oc, :],
                            start=(hh == 0), stop=(hh == NHL - 1))
                    if oc % 2 == 0:
                        nc.scalar.copy(
                            out=o_sb[:, oc * 512:(oc + 1) * 512], in_=o_ps)
                    else:
                        nc.vector.tensor_copy(
                            out=o_sb[:, oc * 512:(oc + 1) * 512], in_=o_ps)
                    if oc == 1:
                        nc.sync.dma_start(
                            out=out[t0:t0 + 128, 0:1024], in_=o_sb[:, 0:1024])
                nc.sync.dma_start(
                    out=out[t0:t0 + 128, 1024:2048], in_=o_sb[:, 1024:2048])

            for c in range(NCH):
                for hh in range(NHL):
                    if c == 0 and hh < 2:
                        continue
                    prev = attn_item(c, hh, prev)
                    if c > 0:
                        proj_item(4 * (c - 1) + hh)
            av_group(*prev)
            for tb in range(4 * (NCH - 1), TB):
                proj_item(tb)

    nc.compile()
    return nc


_NC_CACHE = {}


def get_nc():
    if "nc" not in _NC_CACHE:
        _NC_CACHE["nc"] = build()
    return _NC_CACHE["nc"]


def _tileT(a):
    """[T, X] f32 -> [128, TB, X] bf16 tiled for per-partition-contiguous DMA."""
    Tn, X = a.shape
    return np.ascontiguousarray(
        a.reshape(TB, 128, X).transpose(1, 0, 2).astype(BF))


def make_in_maps(x, ve, cos, sin, Wq, Wk, Wv, Wproj, Wgate):
    """Shard full inputs into 8 per-core input maps (2 batch x 4 head groups)."""
    x = np.asarray(x, np.float32)
    ve = np.asarray(ve, np.float32)
    cosn = np.ascontiguousarray(np.asarray(cos, np.float32)[0, :, 0, :])
    sinn = np.ascontiguousarray(np.asarray(sin, np.float32)[0, :, 0, :])
    Wq = np.asarray(Wq, np.float32)
    Wk = np.asarray(Wk, np.float32)
    Wv = np.asarray(Wv, np.float32)
    Wproj = np.asarray(Wproj, np.float32)
    Wgate = np.asarray(Wgate, np.float32)
    cos_t = _tileT(cosn)
    sin_t = _tileT(sinn)
    in_maps = []
    for core in range(8):
        b, g = divmod(core, 4)
        xb = x[b]
        xtc = np.ascontiguousarray(
            xb.reshape(TB, 128, CT, 128).transpose(3, 0, 2, 1).astype(BF))
        wq_t = np.ascontiguousarray(
            Wq[:, g * 512:(g + 1) * 512]
            .reshape(CT, 128, 512).transpose(1, 0, 2).astype(BF))
        wvx = np.zeros((C, 2 * HD + 1), np.float32)
        wvx[:, 0:HD] = Wk[:, g * 128:(g + 1) * 128]
        wvx[:, HD:2 * HD] = Wv[:, g * 128:(g + 1) * 128]
        wvx[0:GC, 2 * HD] = Wgate[:, g]
        wkv_t = np.ascontiguousarray(
            wvx.reshape(CT, 128, 2 * HD + 1).transpose(1, 0, 2).astype(BF))
        wp = Wproj[g * 512:(g + 1) * 512, :]
        wp_t = np.ascontiguousarray(
            wp.reshape(NHL, 128, OC, 512).transpose(1, 0, 2, 3).astype(BF))
        in_maps.append({
            "xt": xtc,
            "wq": wq_t,
            "wkv": wkv_t,
            "wproj": wp_t,
            "ve": _tileT(2.0 * ve[b][:, g * 128:(g + 1) * 128]),
            "cosn": cos_t,
            "sinn": sin_t,
        })
    return in_maps


def run_cores(in_maps, trace=False, **kw):
    nc = get_nc()
    return run_bass_kernel_spmd(nc, in_maps, core_ids=list(range(8)), trace=trace, **kw)


def kernel(**inputs):
    in_maps = make_in_maps(
        inputs["x"], inputs["ve"], inputs["cos"], inputs["sin"],
        inputs["Wq"], inputs["Wk"], inputs["Wv"], inputs["Wproj"], inputs["Wgate"])
    res = run_cores(in_maps)
    parts = [res.results[i]["out"] for i in range(8)]
    out = np.stack([
        parts[0] + parts[1] + parts[2] + parts[3],
        parts[4] + parts[5] + parts[6] + parts[7],
    ]).astype(np.float32)
    return out


# revision 17
# speedup vs baseline: 1.0069x; 1.0069x over previous
"""Trainium2 Bass kernel for a causal self-attention block (GQA + gated value
embedding + RoPE + QK-RMSNorm), sharded over 8 NeuronCores.

Sharding: 8 cores = 2 (batch) x 4 (kv-head groups).  Each core computes, for
its batch b and head-group g (4 q-heads + 1 kv-head):
    q/k/v projections, gated ve addition, RoPE, RMSNorm, causal attention,
    and the partial output projection  y_g @ Wproj[g*512:(g+1)*512, :].
The host sums the 4 per-group partials for each batch (the Wproj
contraction distributes over head groups).

All matmul operands are bf16 (PSUM accumulation stays fp32); the host
pre-transposes x so no on-device x transposes are needed; attention
S / AV / softmax-denominator run in one merged pipeline with the output
projection interleaved per chunk so the PE stream stays dense.

Self-contained: hardcodes shapes; accepts FULL inputs, returns FULL output.
"""

from contextlib import ExitStack

import ml_dtypes
import numpy as np

import concourse.bacc as bacc
import concourse.bass as bass
import concourse.mybir as mybir
import concourse.tile as tile
from concourse.bass_utils import run_bass_kernel_spmd
from concourse.masks import make_identity

F32 = mybir.dt.float32
BF16 = mybir.dt.bfloat16
I32 = mybir.dt.int32
AF = mybir.ActivationFunctionType
ALU = mybir.AluOpType
AX = mybir.AxisListType
BF = ml_dtypes.bfloat16

B, C, HD, NHL, GC = 2, 2048, 128, 4, 32  # NHL = local q heads per core
T = 2048
TB = T // 128   # token blocks
CT = C // 128   # contraction tiles for qkv
NCH = T // 512  # i-chunks for attention
OC = C // 512   # output chunks for proj
ISQ = 1.0 / float(np.sqrt(128.0))
S128 = float(np.sqrt(128.0))
QKS = 64.0  # host pre-scale on Wq/Wk so fp8 weights sit in the normal range
RSQRT_MAGIC = 0x5F3759DF
F8 = mybir.dt.float8e4
F8NP = ml_dtypes.float8_e4m3
DR = mybir.MatmulPerfMode.DoubleRow


def _bcast(ap_, idx, count):
    """Insert a step-0 (broadcast) dim at position idx of the AP dims."""
    lst = [list(p) for p in ap_.ap]
    lst.insert(idx, [0, count])
    return bass.AP(ap_.tensor, ap_.offset, lst)


def build():
    nc = bacc.Bacc("TRN2", target_bir_lowering=False, debug=False)
    xt = nc.dram_tensor("xt", [128, TB, CT, 128], BF16, kind="ExternalInput")
    wq = nc.dram_tensor("wq", [128, CT, 512], BF16, kind="ExternalInput")
    wkv = nc.dram_tensor("wkv", [128, CT, 2 * HD + 1], BF16, kind="ExternalInput")
    wproj = nc.dram_tensor("wproj", [128, NHL, OC, 512], BF16, kind="ExternalInput")
    ve = nc.dram_tensor("ve", [128, TB, HD], BF16, kind="ExternalInput")
    cosn = nc.dram_tensor("cosn", [128, TB, 64], BF16, kind="ExternalInput")
    sinn = nc.dram_tensor("sinn", [128, TB, 64], BF16, kind="ExternalInput")
    out = nc.dram_tensor("out", [T, C], F32, kind="ExternalOutput")

    with ExitStack() as stk:
        tc = stk.enter_context(tile.TileContext(nc))
        gpool = stk.enter_context(tc.tile_pool(name="gconst", bufs=1))
        warmsrc = gpool.tile([128, 128], BF16)
        nc.vector.memset(warmsrc, 1.0)
        ident = gpool.tile([128, 128], F32)
        make_identity(nc, ident)
        identb = gpool.tile([128, 128], BF16)
        nc.vector.tensor_copy(out=identb, in_=ident)
        ones_f = gpool.tile([128, 128], F32)
        nc.vector.memset(ones_f, 1.0)
        onesb = gpool.tile([128, 128], BF16)
        nc.vector.tensor_copy(out=onesb, in_=ones_f)
        # preload the Exp activation table while DMAs land
        tl_i = gpool.tile([128, 1], F32)
        nc.vector.memset(tl_i, 0.0)
        tl_o = gpool.tile([128, 1], F32)
        nc.scalar.activation(tl_o, tl_i, AF.Exp, scale=-1.0)

        persist = stk.enter_context(tc.tile_pool(name="persist", bufs=1))
        qkT = persist.tile([128, NHL + 1, T], BF16)  # [d, h, t]; h=4 is k
        vS = persist.tile([128, TB, HD], BF16)       # [t%128, t//128, d]
        yT = persist.tile([128, NHL, T], BF16)       # [d, h, t]

        wA = stk.enter_context(tc.tile_pool(name="wA", bufs=1))
        wq_sb = wA.tile([128, CT, 512], BF16)
        wkv_sb = wA.tile([128, CT, 2 * HD + 1], BF16)
        wp_sb = wA.tile([128, NHL, OC, 512], BF16)
        ve_sb = wA.tile([128, TB, HD], BF16)
        cos_sb = wA.tile([128, TB, 64], BF16)
        sin_sb = wA.tile([128, TB, 64], BF16)

        # DMA kickoff, interleaved across the two DGE queues so the first
        # qkv block's operands all land within a few us.
        xns = {}
        xpool = stk.enter_context(tc.tile_pool(name="xpool", bufs=5))
        for tb in range(4):
            xns[tb] = xpool.tile([128, CT, 128], BF16, tag="xn", name="xn")
        nc.gpsimd.dma_start(out=xns[0], in_=xt[:, 0])
        nc.sync.dma_start(out=wq_sb[:, 0:8, :], in_=wq[:, 0:8, :])
        nc.gpsimd.dma_start(out=wq_sb[:, 8:16, :], in_=wq[:, 8:16, :])
        nc.sync.dma_start(out=wkv_sb[:, 0:8, :], in_=wkv[:, 0:8, :])
        nc.gpsimd.dma_start(out=wkv_sb[:, 8:16, :], in_=wkv[:, 8:16, :])
        nc.sync.dma_start(out=xns[1], in_=xt[:, 1])
        nc.gpsimd.dma_start(out=xns[2], in_=xt[:, 2])
        nc.sync.dma_start(out=ve_sb, in_=ve[:, :, :])
        nc.gpsimd.dma_start(out=cos_sb, in_=cosn[:, :, :])
        nc.sync.dma_start(out=xns[3], in_=xt[:, 3])
        nc.gpsimd.dma_start(out=sin_sb, in_=sinn[:, :, :])
        nc.gpsimd.dma_start(out=wp_sb, in_=wproj[:, :, :, :])

        # PE warmup so HAM ramps toward full clock while the DMAs land.
        with tc.tile_pool(name="warm", bufs=2, space="PSUM") as warm:
            for _ in range(24):
                w_ps = warm.tile([128, 128], BF16, tag="wps", name="wps")
                nc.tensor.transpose(w_ps, warmsrc, warmsrc)

        # staged PSUM scopes: qkv-accumulation pools close before the
        # attention pools open; the transpose pool stays open into the
        # attention phase so the last block's transposes can be emitted
        # after the first attention items (PE never waits on the tail
        # of the phase-A DVE chain).
        psA1 = ExitStack()
        psq = psA1.enter_context(tc.tile_pool(name="psq", bufs=3, space="PSUM"))
        pskv = psA1.enter_context(tc.tile_pool(name="pskv", bufs=3, space="PSUM"))
        psA2 = ExitStack()
        pst = psA2.enter_context(
            tc.tile_pool(name="pst", bufs=2, space="PSUM", side="right"))

        sbA = stk.enter_context(tc.tile_pool(name="sbA", bufs=2))
        qkh = stk.enter_context(tc.tile_pool(name="qkh", bufs=3))

        def emit_transposes(pend):
            pqk, pt0 = pend
            for hh in range(NHL + 1):
                tq_ps = pst.tile([128, 128], BF16, tag="tps")
                nc.tensor.transpose(
                    tq_ps, pqk[:, hh * 128:(hh + 1) * 128], identb)
                nc.scalar.copy(out=qkT[:, hh, pt0:pt0 + 128], in_=tq_ps)

        # ---------------- phase A: qkv + rope + rmsnorm + transposes --------
        with nc.named_scope("phaseA"):
            pends = []  # (qkhat, t0) awaiting transpose into qkT, lag 2
            for tb in range(TB):
                t0 = tb * 128
                xn = xns.pop(tb)
                q_ps = psq.tile([128, NHL * HD], F32, tag="qps")
                kv_ps = pskv.tile([128, 2 * HD + 1], F32, tag="kvps")

                # qkv matmuls (k, v and the gate column fused in one rhs);
                # transposes of the block-before-last are interleaved
                # (2-block lag gives the DVE chain time to finish, so the
                # PE never waits on qkhat)
                pend = pends.pop(0) if len(pends) == 2 else None
                for ct in range(CT):
                    nc.tensor.matmul(
                        q_ps, lhsT=xn[:, ct, :], rhs=wq_sb[:, ct, :],
                        start=(ct == 0), stop=(ct == CT - 1))
                    nc.tensor.matmul(
                        kv_ps, lhsT=xn[:, ct, :], rhs=wkv_sb[:, ct, :],
                        start=(ct == 0), stop=(ct == CT - 1))
                    if pend is not None and ct in (2, 5, 8, 11, 14):
                        hh = (ct - 2) // 3
                        pqk, pt0 = pend
                        tq_ps = pst.tile([128, 128], BF16, tag="tps")
                        nc.tensor.transpose(
                            tq_ps, pqk[:, hh * 128:(hh + 1) * 128], identb)
                        nc.scalar.copy(
                            out=qkT[:, hh, pt0:pt0 + 128], in_=tq_ps)

                # prefetch: emitted after this block's matmuls so the ring
                # slot's previous reads are already ordered before the write
                if tb + 4 < TB:
                    xpf = xpool.tile([128, CT, 128], BF16, tag="xn", name="xn")
                    nc.sync.dma_start(out=xpf, in_=xt[:, tb + 4])
                    xns[tb + 4] = xpf

                # PSUM -> SBUF casts: q(4 heads)+k into one 5-head rope tile
                qkb = qkh.tile([128, (NHL + 1) * HD], BF16, tag="qkb")
                nc.scalar.copy(out=qkb[:, 0:512], in_=q_ps)
                nc.scalar.copy(out=qkb[:, 512:640], in_=kv_ps[:, 0:HD])
                vb = sbA.tile([128, HD], BF16, tag="vb")
                nc.scalar.copy(out=vb, in_=kv_ps[:, HD:2 * HD])

                # gate = sigmoid(z); z rides the kv matmul as weight col 256
                e_sb = sbA.tile([128, 1], F32, tag="esb")
                nc.scalar.activation(
                    e_sb, kv_ps[:, 2 * HD:2 * HD + 1], AF.Exp, scale=-1.0)
                nc.vector.tensor_scalar_add(e_sb, e_sb, 1.0)
                g_sb = sbA.tile([128, 1], F32, tag="gsb")
                nc.vector.reciprocal(g_sb, e_sb)
                # v = v_mm + sigmoid(z) * (2*ve)
                nc.vector.scalar_tensor_tensor(
                    out=vS[:, tb, :], in0=ve_sb[:, tb, :], scalar=g_sb,
                    in1=vb, op0=ALU.mult, op1=ALU.add)

                # ---- RoPE on q (4 heads) and k batched as 5 heads ----
                NH5 = NHL + 1
                cosB = _bcast(cos_sb[:, tb, :], 1, NH5)
                sinB = _bcast(sin_sb[:, tb, :], 1, NH5)
                qv = qkb.rearrange("p (h d) -> p h d", h=NH5)
                rh = sbA.tile([128, NH5 * HD], BF16, tag="rh")
                rhv = rh.rearrange("p (h d) -> p h d", h=NH5)
                tmp = sbA.tile([128, NH5, 64], BF16, tag="tmp")
                nc.vector.tensor_tensor(
                    out=rhv[:, :, 0:64], in0=qv[:, :, 0:64], in1=cosB,
                    op=ALU.mult)
                nc.vector.tensor_tensor(
                    out=tmp, in0=qv[:, :, 64:128], in1=sinB, op=ALU.mult)
                nc.vector.tensor_tensor(
                    out=rhv[:, :, 0:64], in0=rhv[:, :, 0:64], in1=tmp,
                    op=ALU.add)
                nc.vector.tensor_tensor(
                    out=rhv[:, :, 64:128], in0=qv[:, :, 64:128], in1=cosB,
                    op=ALU.mult)
                nc.vector.tensor_tensor(
                    out=tmp, in0=qv[:, :, 0:64], in1=sinB, op=ALU.mult)
                nc.vector.tensor_tensor(
                    out=rhv[:, :, 64:128], in0=rhv[:, :, 64:128], in1=tmp,
                    op=ALU.subtract)

                # ---- RMSNorm scales for 5 heads in one [128, 5] batch ----
                # rq = sqrt(128)*rsqrt(sum(q^2)) = rsqrt(mean(q^2)); the
                # sqrt(128) is folded into the Newton-step constants.
                sq2 = sbA.tile([128, NH5 * HD], BF16, tag="sq2")
                nc.vector.tensor_tensor(out=sq2, in0=rh, in1=rh, op=ALU.mult)
                red = sbA.tile([128, NH5], F32, tag="red")
                nc.vector.tensor_reduce(
                    out=red, in_=sq2.rearrange("p (h d) -> p h d", h=NH5),
                    axis=AX.X, op=ALU.add)
                rq = sbA.tile([128, NH5], F32, tag="rq")
                rqi = rq.bitcast(I32)
                nc.vector.tensor_scalar(
                    out=rqi, in0=red.bitcast(I32), scalar1=1, scalar2=None,
                    op0=ALU.logical_shift_right)
                nc.vector.tensor_scalar(
                    out=rqi, in0=rqi, scalar1=-1, scalar2=RSQRT_MAGIC,
                    op0=ALU.mult, op1=ALU.add)
                nt = sbA.tile([128, NH5], F32, tag="nt")
                nc.vector.tensor_tensor(out=nt, in0=rq, in1=rq, op=ALU.mult)
                nc.vector.tensor_tensor(out=nt, in0=nt, in1=red, op=ALU.mult)
                nc.vector.tensor_scalar(
                    out=nt, in0=nt, scalar1=-0.5 * S128, scalar2=1.5 * S128,
                    op0=ALU.mult, op1=ALU.add)
                nc.vector.tensor_tensor(out=rq, in0=rq, in1=nt, op=ALU.mult)

                qkhat = qkh.tile([128, NH5 * HD], BF16, tag="qkhat")
                for h5 in range(NH5):
                    nc.vector.tensor_scalar_mul(
                        qkhat[:, h5 * HD:(h5 + 1) * HD],
                        rhv[:, h5, :], rq[:, h5:h5 + 1])
                pends.append((qkhat, t0))

        psA1.close()  # free qkv PSUM banks for the attention pools

        # ---------------- phase B+C: attention + output projection ---------
        # c-outer / head-inner, software-pipelined S/exp one group ahead of
        # AV; softmax denominator via per-group DVE fold + one matmul; the
        # output projection for chunk c-1's four token blocks is interleaved
        # between heads of chunk c so the PE stream stays dense end-to-end.
        ptB = stk.enter_context(tc.tile_pool(name="ptB", bufs=6))
        smB = stk.enter_context(tc.tile_pool(name="smB", bufs=2))
        osb = stk.enter_context(tc.tile_pool(name="osb", bufs=2))
        with nc.named_scope("phaseBC"):
            psS = stk.enter_context(tc.tile_pool(name="psS", bufs=3, space="PSUM"))
            psy = stk.enter_context(tc.tile_pool(name="psy", bufs=2, space="PSUM"))
            psd = stk.enter_context(tc.tile_pool(name="psd", bufs=1, space="PSUM"))

            def s_group(meta, g):
                hh, c, i0 = meta["hh"], meta["c"], meta["i0"]
                pt = ptB.tile([128, 1024], BF16, tag="pt", name="pt")
                for s in range(2):
                    jb = 2 * g + s
                    io = max(0, 128 * jb - 512 * c)  # first causally-live col
                    sps = psS.tile([128, 512], F32, tag="sps", name="sps")
                    nc.tensor.matmul(
                        sps[:, io:512],
                        lhsT=qkT[:, NHL, jb * 128:(jb + 1) * 128],
                        rhs=qkT[:, hh, i0 + io:i0 + 512],
                        start=True, stop=True)
                    nc.scalar.activation(
                        pt[:, s * 512 + io:(s + 1) * 512],
                        sps[:, io:512], AF.Exp, scale=ISQ)
                for s in range(2):
                    jb = 2 * g + s
                    if jb >= 4 * c:  # diagonal block: zero j > i
                        io = 128 * (jb - 4 * c)
                        nc.gpsimd.affine_select(
                            out=pt[:, s * 512 + io:(s + 1) * 512],
                            in_=pt[:, s * 512 + io:(s + 1) * 512],
                            pattern=[[1, 512 - io]], compare_op=ALU.is_ge,
                            fill=0.0, base=0, channel_multiplier=-1)
                meta["pts"][g] = pt

            def av_group(meta, g, is_last):
                pt = meta["pts"].pop(g)
                yps = meta["yps"]
                c = meta["c"]
                for s in range(2):
                    jb = 2 * g + s
                    io = max(0, 128 * jb - 512 * c)
                    if jb == meta["first_jb"]:
                        io = 0  # start matmul must cover the full chunk
                    nc.tensor.matmul(
                        yps[:, io:512], lhsT=vS[:, jb, :],
                        rhs=pt[:, s * 512 + io:(s + 1) * 512],
                        start=(jb == meta["first_jb"]),
                        stop=(jb == meta["last_jb"]))
                # fold the two key blocks of this group on DVE, then one
                # per-group denominator matmul (halves the PE denominator)
                dps = meta["dps"]
                io0 = max(0, 128 * (2 * g) - 512 * c)
                io1 = max(0, 128 * (2 * g + 1) - 512 * c)
                acc = smB.tile([128, 512], BF16, tag="acc", name="acc")
                nc.vector.tensor_tensor(
                    out=acc[:, io1:512], in0=pt[:, io1:512],
                    in1=pt[:, 512 + io1:1024], op=ALU.add)
                if io1 > io0:
                    nc.vector.tensor_copy(
                        out=acc[:, io0:io1], in_=pt[:, io0:io1])
                nc.tensor.matmul(
                    dps[:, io0:512], lhsT=onesb, rhs=acc[:, io0:512],
                    start=(g == meta["order0"]),
                    stop=(g == meta["order_last"]))
                if is_last:
                    hh, i0 = meta["hh"], meta["i0"]
                    rcp = smB.tile([128, 512], F32, tag="rcp", name="rcp")
                    nc.vector.reciprocal_approx_fast(out=rcp, in_=dps)
                    nc.vector.tensor_tensor(
                        out=yT[:, hh, i0:i0 + 512], in0=yps, in1=rcp,
                        op=ALU.mult)

            def attn_item(c, hh, prev):
                yps = psy.tile([128, 512], F32, tag="yps", name="yps")
                dps = psd.tile([128, 512], F32, tag="dps", name="dps")
                ngrp = (4 * c + 4) // 2
                order = list(range(ngrp - 2, ngrp)) + list(range(ngrp - 2))
                meta = dict(hh=hh, c=c, i0=c * 512, yps=yps, dps=dps,
                            pts={}, first_jb=2 * order[0],
                            last_jb=2 * order[-1] + 1,
                            order0=order[0], order_last=order[-1])
                for idx, g in enumerate(order):
                    s_group(meta, g)
                    if prev is not None:
                        av_group(*prev)
                    prev = (meta, g, idx == len(order) - 1)
                return prev

            # first attention items run while the last qkv blocks' DVE
            # chains finish; their transposes are emitted in between.
            prev = attn_item(0, 0, None)
            emit_transposes(pends[0])
            prev = attn_item(0, 1, prev)
            emit_transposes(pends[1])
            psA2.close()  # free the transpose PSUM banks
            psC = stk.enter_context(tc.tile_pool(name="psC", bufs=2, space="PSUM"))

            def proj_item(tb):
                t0 = tb * 128
                o_sb = osb.tile([128, C], F32, tag="osb", name="osb")
                for oc in range(OC):
                    o_ps = psC.tile([128, 512], F32, tag="ops", name="ops")
                    for hh in range(NHL):
                        nc.tensor.matmul(
                            o_ps, lhsT=yT[:, hh, t0:t0 + 128],
                            rhs=wp_sb[:, hh, oc, :],
                            start=(hh == 0), stop=(hh == NHL - 1))
                    if oc % 2 == 0:
                        nc.scalar.copy(
                            out=o_sb[:, oc * 512:(oc + 1) * 512], in_=o_ps)
                    else:
                        nc.vector.tensor_copy(
                            out=o_sb[:, oc * 512:(oc + 1) * 512], in_=o_ps)
                    if oc == 1:
                        nc.sync.dma_start(
                            out=out[t0:t0 + 128, 0:1024], in_=o_sb[:, 0:1024])
                nc.sync.dma_start(
                    out=out[t0:t0 + 128, 1024:2048], in_=o_sb[:, 1024:2048])

            for c in range(NCH):
                for hh in range(NHL):
                    if c == 0 and hh < 2:
                        continue
                    prev = attn_item(c, hh, prev)
                    if c > 0:
                        proj_item(4 * (c - 1) + hh)
            av_group(*prev)
            for tb in range(4 * (NCH - 1), TB):
                proj_item(tb)

    nc.compile()
    return nc


_NC_CACHE = {}


def get_nc():
    if "nc" not in _NC_CACHE:
        _NC_CACHE["nc"] = build()
    return _NC_CACHE["nc"]


def _tileT(a):
    """[T, X] f32 -> [128, TB, X] bf16 tiled for per-partition-contiguous DMA."""
    Tn, X = a.shape
    return np.ascontiguousarray(
        a.reshape(TB, 128, X).transpose(1, 0, 2).astype(BF))


def make_in_maps(x, ve, cos, sin, Wq, Wk, Wv, Wproj, Wgate):
    """Shard full inputs into 8 per-core input maps (2 batch x 4 head groups)."""
    x = np.asarray(x, np.float32)
    ve = np.asarray(ve, np.float32)
    cosn = np.ascontiguousarray(np.asarray(cos, np.float32)[0, :, 0, :])
    sinn = np.ascontiguousarray(np.asarray(sin, np.float32)[0, :, 0, :])
    Wq = np.asarray(Wq, np.float32)
    Wk = np.asarray(Wk, np.float32)
    Wv = np.asarray(Wv, np.float32)
    Wproj = np.asarray(Wproj, np.float32)
    Wgate = np.asarray(Wgate, np.float32)
    cos_t = _tileT(cosn)
    sin_t = _tileT(sinn)
    in_maps = []
    for core in range(8):
        b, g = divmod(core, 4)
        xb = x[b]
        xtc = np.ascontiguousarray(
            xb.reshape(TB, 128, CT, 128).transpose(3, 0, 2, 1).astype(BF))
        wq_t = np.ascontiguousarray(
            Wq[:, g * 512:(g + 1) * 512]
            .reshape(CT, 128, 512).transpose(1, 0, 2).astype(BF))
        wvx = np.zeros((C, 2 * HD + 1), np.float32)
        wvx[:, 0:HD] = Wk[:, g * 128:(g + 1) * 128]
        wvx[:, HD:2 * HD] = Wv[:, g * 128:(g + 1) * 128]
        wvx[0:GC, 2 * HD] = Wgate[:, g]
        wkv_t = np.ascontiguousarray(
            wvx.reshape(CT, 128, 2 * HD + 1).transpose(1, 0, 2).astype(BF))
        wp = Wproj[g * 512:(g + 1) * 512, :]
        wp_t = np.ascontiguousarray(
            wp.reshape(NHL, 128, OC, 512).transpose(1, 0, 2, 3).astype(BF))
        in_maps.append({
            "xt": xtc,
            "wq": wq_t,
            "wkv": wkv_t,
            "wproj": wp_t,
            "ve": _tileT(2.0 * ve[b][:, g * 128:(g + 1) * 128]),
            "cosn": cos_t,
            "sinn": sin_t,
        })
    return in_maps


def run_cores(in_maps, trace=False, **kw):
    nc = get_nc()
    return run_bass_kernel_spmd(nc, in_maps, core_ids=list(range(8)), trace=trace, **kw)


def kernel(**inputs):
    in_maps = make_in_maps(
        inputs["x"], inputs["ve"], inputs["cos"], inputs["sin"],
        inputs["Wq"], inputs["Wk"], inputs["Wv"], inputs["Wproj"], inputs["Wgate"])
    res = run_cores(in_maps)
    parts = [res.results[i]["out"] for i in range(8)]
    out = np.stack([
        parts[0] + parts[1] + parts[2] + parts[3],
        parts[4] + parts[5] + parts[6] + parts[7],
    ]).astype(np.float32)
    return out


# revision 19
# speedup vs baseline: 1.0096x; 1.0027x over previous
"""Trainium2 Bass kernel for a causal self-attention block (GQA + gated value
embedding + RoPE + QK-RMSNorm), sharded over 8 NeuronCores.

Sharding: 8 cores = 2 (batch) x 4 (kv-head groups).  Each core computes, for
its batch b and head-group g (4 q-heads + 1 kv-head):
    q/k/v projections, gated ve addition, RoPE, RMSNorm, causal attention,
    and the partial output projection  y_g @ Wproj[g*512:(g+1)*512, :].
The host sums the 4 per-group partials for each batch (the Wproj
contraction distributes over head groups).

All matmul operands are bf16 (PSUM accumulation stays fp32); the host
pre-transposes x so no on-device x transposes are needed; attention
S / AV / softmax-denominator run in one merged pipeline with the output
projection interleaved per chunk so the PE stream stays dense.

Self-contained: hardcodes shapes; accepts FULL inputs, returns FULL output.
"""

from contextlib import ExitStack

import ml_dtypes
import numpy as np

import concourse.bacc as bacc
import concourse.bass as bass
import concourse.mybir as mybir
import concourse.tile as tile
from concourse.bass_utils import run_bass_kernel_spmd
from concourse.masks import make_identity

F32 = mybir.dt.float32
BF16 = mybir.dt.bfloat16
I32 = mybir.dt.int32
AF = mybir.ActivationFunctionType
ALU = mybir.AluOpType
AX = mybir.AxisListType
BF = ml_dtypes.bfloat16

B, C, HD, NHL, GC = 2, 2048, 128, 4, 32  # NHL = local q heads per core
T = 2048
TB = T // 128   # token blocks
CT = C // 128   # contraction tiles for qkv
NCH = T // 512  # i-chunks for attention
OC = C // 512   # output chunks for proj
ISQ = 1.0 / float(np.sqrt(128.0))
S128 = float(np.sqrt(128.0))
QKS = 64.0  # host pre-scale on Wq/Wk so fp8 weights sit in the normal range
RSQRT_MAGIC = 0x5F3759DF
F8 = mybir.dt.float8e4
F8NP = ml_dtypes.float8_e4m3
DR = mybir.MatmulPerfMode.DoubleRow


def _bcast(ap_, idx, count):
    """Insert a step-0 (broadcast) dim at position idx of the AP dims."""
    lst = [list(p) for p in ap_.ap]
    lst.insert(idx, [0, count])
    return bass.AP(ap_.tensor, ap_.offset, lst)


def build():
    nc = bacc.Bacc("TRN2", target_bir_lowering=False, debug=False)
    xt = nc.dram_tensor("xt", [128, TB, CT, 128], BF16, kind="ExternalInput")
    wq = nc.dram_tensor("wq", [128, CT, 512], BF16, kind="ExternalInput")
    wkv = nc.dram_tensor("wkv", [128, CT, 2 * HD + 1], BF16, kind="ExternalInput")
    wproj = nc.dram_tensor("wproj", [128, NHL, OC, 512], BF16, kind="ExternalInput")
    ve = nc.dram_tensor("ve", [128, TB, HD], BF16, kind="ExternalInput")
    cosn = nc.dram_tensor("cosn", [128, TB, 64], BF16, kind="ExternalInput")
    sinn = nc.dram_tensor("sinn", [128, TB, 64], BF16, kind="ExternalInput")
    out = nc.dram_tensor("out", [T, C], F32, kind="ExternalOutput")

    with ExitStack() as stk:
        tc = stk.enter_context(tile.TileContext(nc))
        gpool = stk.enter_context(tc.tile_pool(name="gconst", bufs=1))
        warmsrc = gpool.tile([128, 128], BF16)
        nc.vector.memset(warmsrc, 1.0)
        ident = gpool.tile([128, 128], F32)
        make_identity(nc, ident)
        identb = gpool.tile([128, 128], BF16)
        nc.vector.tensor_copy(out=identb, in_=ident)
        ones_f = gpool.tile([128, 128], F32)
        nc.vector.memset(ones_f, 1.0)
        onesb = gpool.tile([128, 128], BF16)
        nc.vector.tensor_copy(out=onesb, in_=ones_f)
        # preload the Exp activation table while DMAs land
        tl_i = gpool.tile([128, 1], F32)
        nc.vector.memset(tl_i, 0.0)
        tl_o = gpool.tile([128, 1], F32)
        nc.scalar.activation(tl_o, tl_i, AF.Exp, scale=-1.0)

        persist = stk.enter_context(tc.tile_pool(name="persist", bufs=1))
        qkT = persist.tile([128, NHL + 1, T], BF16)  # [d, h, t]; h=4 is k
        vS = persist.tile([128, TB, HD], BF16)       # [t%128, t//128, d]
        yT = persist.tile([128, NHL, T], BF16)       # [d, h, t]

        wA = stk.enter_context(tc.tile_pool(name="wA", bufs=1))
        wq_sb = wA.tile([128, CT, 512], BF16)
        wkv_sb = wA.tile([128, CT, 2 * HD + 1], BF16)
        wp_sb = wA.tile([128, NHL, OC, 512], BF16)
        ve_sb = wA.tile([128, TB, HD], BF16)
        cos_sb = wA.tile([128, TB, 64], BF16)
        sin_sb = wA.tile([128, TB, 64], BF16)

        # DMA kickoff, interleaved across the two DGE queues so the first
        # qkv block's operands all land within a few us.
        xns = {}
        xpool = stk.enter_context(tc.tile_pool(name="xpool", bufs=5))
        for tb in range(4):
            xns[tb] = xpool.tile([128, CT, 128], BF16, tag="xn", name="xn")
        nc.gpsimd.dma_start(out=xns[0], in_=xt[:, 0])
        nc.sync.dma_start(out=wq_sb[:, 0:8, :], in_=wq[:, 0:8, :])
        nc.gpsimd.dma_start(out=wq_sb[:, 8:16, :], in_=wq[:, 8:16, :])
        nc.sync.dma_start(out=wkv_sb[:, 0:8, :], in_=wkv[:, 0:8, :])
        nc.gpsimd.dma_start(out=wkv_sb[:, 8:16, :], in_=wkv[:, 8:16, :])
        nc.sync.dma_start(out=xns[1], in_=xt[:, 1])
        nc.gpsimd.dma_start(out=xns[2], in_=xt[:, 2])
        nc.sync.dma_start(out=ve_sb, in_=ve[:, :, :])
        nc.gpsimd.dma_start(out=cos_sb, in_=cosn[:, :, :])
        nc.sync.dma_start(out=xns[3], in_=xt[:, 3])
        nc.gpsimd.dma_start(out=sin_sb, in_=sinn[:, :, :])
        nc.gpsimd.dma_start(out=wp_sb, in_=wproj[:, :, :, :])

        # PE warmup so HAM ramps toward full clock while the DMAs land.
        with tc.tile_pool(name="warm", bufs=2, space="PSUM") as warm:
            for _ in range(24):
                w_ps = warm.tile([128, 128], BF16, tag="wps", name="wps")
                nc.tensor.transpose(w_ps, warmsrc, warmsrc)

        # staged PSUM scopes: qkv-accumulation pools close before the
        # attention pools open; the transpose pool stays open into the
        # attention phase so the last block's transposes can be emitted
        # after the first attention items (PE never waits on the tail
        # of the phase-A DVE chain).
        psA1 = ExitStack()
        psq = psA1.enter_context(tc.tile_pool(name="psq", bufs=3, space="PSUM"))
        pskv = psA1.enter_context(tc.tile_pool(name="pskv", bufs=3, space="PSUM"))
        psA2 = ExitStack()
        pst = psA2.enter_context(
            tc.tile_pool(name="pst", bufs=2, space="PSUM", side="right"))

        sbA = stk.enter_context(tc.tile_pool(name="sbA", bufs=2))
        qkh = stk.enter_context(tc.tile_pool(name="qkh", bufs=3))

        def emit_transposes(pend):
            pqk, pt0 = pend
            for hh in range(NHL + 1):
                tq_ps = pst.tile([128, 128], BF16, tag="tps")
                nc.tensor.transpose(
                    tq_ps, pqk[:, hh * 128:(hh + 1) * 128], identb)
                nc.scalar.copy(out=qkT[:, hh, pt0:pt0 + 128], in_=tq_ps)

        # ---------------- phase A: qkv + rope + rmsnorm + transposes --------
        with nc.named_scope("phaseA"):
            pends = []  # (qkhat, t0) awaiting transpose into qkT, lag 2
            for tb in range(TB):
                t0 = tb * 128
                xn = xns.pop(tb)
                q_ps = psq.tile([128, NHL * HD], F32, tag="qps")
                kv_ps = pskv.tile([128, 2 * HD + 1], F32, tag="kvps")

                # qkv matmuls (k, v and the gate column fused in one rhs);
                # transposes of the block-before-last are interleaved
                # (2-block lag gives the DVE chain time to finish, so the
                # PE never waits on qkhat)
                pend = pends.pop(0) if len(pends) == 2 else None
                for ct in range(CT):
                    nc.tensor.matmul(
                        q_ps, lhsT=xn[:, ct, :], rhs=wq_sb[:, ct, :],
                        start=(ct == 0), stop=(ct == CT - 1))
                    nc.tensor.matmul(
                        kv_ps, lhsT=xn[:, ct, :], rhs=wkv_sb[:, ct, :],
                        start=(ct == 0), stop=(ct == CT - 1))
                    if pend is not None and ct in (2, 5, 8, 11, 14):
                        hh = (ct - 2) // 3
                        pqk, pt0 = pend
                        tq_ps = pst.tile([128, 128], BF16, tag="tps")
                        nc.tensor.transpose(
                            tq_ps, pqk[:, hh * 128:(hh + 1) * 128], identb)
                        nc.scalar.copy(
                            out=qkT[:, hh, pt0:pt0 + 128], in_=tq_ps)

                # prefetch: emitted after this block's matmuls so the ring
                # slot's previous reads are already ordered before the write
                if tb + 4 < TB:
                    xpf = xpool.tile([128, CT, 128], BF16, tag="xn", name="xn")
                    nc.sync.dma_start(out=xpf, in_=xt[:, tb + 4])
                    xns[tb + 4] = xpf

                # PSUM -> SBUF casts: q(4 heads)+k into one 5-head rope tile
                qkb = qkh.tile([128, (NHL + 1) * HD], BF16, tag="qkb")
                nc.scalar.copy(out=qkb[:, 0:512], in_=q_ps)
                nc.scalar.copy(out=qkb[:, 512:640], in_=kv_ps[:, 0:HD])
                vb = sbA.tile([128, HD], BF16, tag="vb")
                nc.scalar.copy(out=vb, in_=kv_ps[:, HD:2 * HD])

                # gate = sigmoid(z); z rides the kv matmul as weight col 256
                e_sb = sbA.tile([128, 1], F32, tag="esb")
                nc.scalar.activation(
                    e_sb, kv_ps[:, 2 * HD:2 * HD + 1], AF.Exp, scale=-1.0)
                nc.vector.tensor_scalar_add(e_sb, e_sb, 1.0)
                g_sb = sbA.tile([128, 1], F32, tag="gsb")
                nc.vector.reciprocal(g_sb, e_sb)
                # v = v_mm + sigmoid(z) * (2*ve)
                nc.vector.scalar_tensor_tensor(
                    out=vS[:, tb, :], in0=ve_sb[:, tb, :], scalar=g_sb,
                    in1=vb, op0=ALU.mult, op1=ALU.add)

                # ---- RoPE on q (4 heads) and k batched as 5 heads ----
                NH5 = NHL + 1
                cosB = _bcast(cos_sb[:, tb, :], 1, NH5)
                sinB = _bcast(sin_sb[:, tb, :], 1, NH5)
                qv = qkb.rearrange("p (h d) -> p h d", h=NH5)
                rh = sbA.tile([128, NH5 * HD], BF16, tag="rh")
                rhv = rh.rearrange("p (h d) -> p h d", h=NH5)
                tmp = sbA.tile([128, NH5, 64], BF16, tag="tmp")
                nc.vector.tensor_tensor(
                    out=rhv[:, :, 0:64], in0=qv[:, :, 0:64], in1=cosB,
                    op=ALU.mult)
                nc.vector.tensor_tensor(
                    out=tmp, in0=qv[:, :, 64:128], in1=sinB, op=ALU.mult)
                nc.vector.tensor_tensor(
                    out=rhv[:, :, 0:64], in0=rhv[:, :, 0:64], in1=tmp,
                    op=ALU.add)
                nc.vector.tensor_tensor(
                    out=rhv[:, :, 64:128], in0=qv[:, :, 64:128], in1=cosB,
                    op=ALU.mult)
                nc.vector.tensor_tensor(
                    out=tmp, in0=qv[:, :, 0:64], in1=sinB, op=ALU.mult)
                nc.vector.tensor_tensor(
                    out=rhv[:, :, 64:128], in0=rhv[:, :, 64:128], in1=tmp,
                    op=ALU.subtract)

                # ---- RMSNorm scales for 5 heads in one [128, 5] batch ----
                # rq = sqrt(128)*rsqrt(sum(q^2)) = rsqrt(mean(q^2)); the
                # sqrt(128) is folded into the Newton-step constants.
                sq2 = sbA.tile([128, NH5 * HD], BF16, tag="sq2")
                nc.vector.tensor_tensor(out=sq2, in0=rh, in1=rh, op=ALU.mult)
                red = sbA.tile([128, NH5], F32, tag="red")
                nc.vector.tensor_reduce(
                    out=red, in_=sq2.rearrange("p (h d) -> p h d", h=NH5),
                    axis=AX.X, op=ALU.add)
                rq = sbA.tile([128, NH5], F32, tag="rq")
                rqi = rq.bitcast(I32)
                nc.vector.tensor_scalar(
                    out=rqi, in0=red.bitcast(I32), scalar1=1, scalar2=None,
                    op0=ALU.logical_shift_right)
                nc.vector.tensor_scalar(
                    out=rqi, in0=rqi, scalar1=-1, scalar2=RSQRT_MAGIC,
                    op0=ALU.mult, op1=ALU.add)
                nt = sbA.tile([128, NH5], F32, tag="nt")
                nc.vector.tensor_tensor(out=nt, in0=rq, in1=rq, op=ALU.mult)
                nc.vector.tensor_tensor(out=nt, in0=nt, in1=red, op=ALU.mult)
                nc.vector.tensor_scalar(
                    out=nt, in0=nt, scalar1=-0.5 * S128, scalar2=1.5 * S128,
                    op0=ALU.mult, op1=ALU.add)
                nc.vector.tensor_tensor(out=rq, in0=rq, in1=nt, op=ALU.mult)

                qkhat = qkh.tile([128, NH5 * HD], BF16, tag="qkhat")
                for h5 in range(NH5):
                    nc.vector.tensor_scalar_mul(
                        qkhat[:, h5 * HD:(h5 + 1) * HD],
                        rhv[:, h5, :], rq[:, h5:h5 + 1])
                pends.append((qkhat, t0))

        psA1.close()  # free qkv PSUM banks for the attention pools

        # ---------------- phase B+C: attention + output projection ---------
        # c-outer / head-inner, software-pipelined S/exp one group ahead of
        # AV; softmax denominator via per-group DVE fold + one matmul; the
        # output projection for chunk c-1's four token blocks is interleaved
        # between heads of chunk c so the PE stream stays dense end-to-end.
        ptB = stk.enter_context(tc.tile_pool(name="ptB", bufs=6))
        smB = stk.enter_context(tc.tile_pool(name="smB", bufs=2))
        osb = stk.enter_context(tc.tile_pool(name="osb", bufs=2))
        with nc.named_scope("phaseBC"):
            psS = stk.enter_context(tc.tile_pool(name="psS", bufs=3, space="PSUM"))
            psy = stk.enter_context(tc.tile_pool(name="psy", bufs=2, space="PSUM"))
            psd = stk.enter_context(tc.tile_pool(name="psd", bufs=1, space="PSUM"))

            def s_group(meta, g):
                hh, c, i0 = meta["hh"], meta["c"], meta["i0"]
                pt = ptB.tile([128, 1024], BF16, tag="pt", name="pt")
                for s in range(2):
                    jb = 2 * g + s
                    io = max(0, 128 * jb - 512 * c)  # first causally-live col
                    sps = psS.tile([128, 512], F32, tag="sps", name="sps")
                    nc.tensor.matmul(
                        sps[:, io:512],
                        lhsT=qkT[:, NHL, jb * 128:(jb + 1) * 128],
                        rhs=qkT[:, hh, i0 + io:i0 + 512],
                        start=True, stop=True)
                    nc.scalar.activation(
                        pt[:, s * 512 + io:(s + 1) * 512],
                        sps[:, io:512], AF.Exp, scale=ISQ)
                for s in range(2):
                    jb = 2 * g + s
                    if jb >= 4 * c:  # diagonal block: zero j > i
                        io = 128 * (jb - 4 * c)
                        nc.gpsimd.affine_select(
                            out=pt[:, s * 512 + io:(s + 1) * 512],
                            in_=pt[:, s * 512 + io:(s + 1) * 512],
                            pattern=[[1, 512 - io]], compare_op=ALU.is_ge,
                            fill=0.0, base=0, channel_multiplier=-1)
                meta["pts"][g] = pt

            def av_group(meta, g, is_last):
                pt = meta["pts"].pop(g)
                yps = meta["yps"]
                c = meta["c"]
                for s in range(2):
                    jb = 2 * g + s
                    io = max(0, 128 * jb - 512 * c)
                    if jb == meta["first_jb"]:
                        io = 0  # start matmul must cover the full chunk
                    nc.tensor.matmul(
                        yps[:, io:512], lhsT=vS[:, jb, :],
                        rhs=pt[:, s * 512 + io:(s + 1) * 512],
                        start=(jb == meta["first_jb"]),
                        stop=(jb == meta["last_jb"]))
                # fold the two key blocks of this group on DVE, then one
                # per-group denominator matmul (halves the PE denominator)
                dps = meta["dps"]
                io0 = max(0, 128 * (2 * g) - 512 * c)
                io1 = max(0, 128 * (2 * g + 1) - 512 * c)
                acc = smB.tile([128, 512], BF16, tag="acc", name="acc")
                nc.vector.tensor_tensor(
                    out=acc[:, io1:512], in0=pt[:, io1:512],
                    in1=pt[:, 512 + io1:1024], op=ALU.add)
                if io1 > io0:
                    nc.vector.tensor_copy(
                        out=acc[:, io0:io1], in_=pt[:, io0:io1])
                nc.tensor.matmul(
                    dps[:, io0:512], lhsT=onesb, rhs=acc[:, io0:512],
                    start=(g == meta["order0"]),
                    stop=(g == meta["order_last"]))
                if is_last:
                    hh, i0 = meta["hh"], meta["i0"]
                    rcp = smB.tile([128, 512], F32, tag="rcp", name="rcp")
                    nc.vector.reciprocal_approx_fast(out=rcp, in_=dps)
                    nc.vector.tensor_tensor(
                        out=yT[:, hh, i0:i0 + 512], in0=yps, in1=rcp,
                        op=ALU.mult)

            def attn_item(c, hh, prev):
                yps = psy.tile([128, 512], F32, tag="yps", name="yps")
                dps = psd.tile([128, 512], F32, tag="dps", name="dps")
                ngrp = (4 * c + 4) // 2
                order = list(range(ngrp - 2, ngrp)) + list(range(ngrp - 2))
                meta = dict(hh=hh, c=c, i0=c * 512, yps=yps, dps=dps,
                            pts={}, first_jb=2 * order[0],
                            last_jb=2 * order[-1] + 1,
                            order0=order[0], order_last=order[-1])
                for idx, g in enumerate(order):
                    s_group(meta, g)
                    if prev is not None:
                        av_group(*prev)
                    prev = (meta, g, idx == len(order) - 1)
                return prev

            # first attention items run while the last qkv blocks' DVE
            # chains finish; their transposes are emitted in between.
            prev = attn_item(0, 0, None)
            emit_transposes(pends[0])
            prev = attn_item(0, 1, prev)
            emit_transposes(pends[1])
            psA2.close()  # free the transpose PSUM banks
            psC = stk.enter_context(tc.tile_pool(name="psC", bufs=2, space="PSUM"))

            def proj_item(tb):
                t0 = tb * 128
                o_sb = osb.tile([128, C], F32, tag="osb", name="osb")
                for oc in range(OC):
                    o_ps = psC.tile([128, 512], F32, tag="ops", name="ops")
                    for hh in range(NHL):
                        nc.tensor.matmul(
                            o_ps, lhsT=yT[:, hh, t0:t0 + 128],
                            rhs=wp_sb[:, hh, oc, :],
                            start=(hh == 0), stop=(hh == NHL - 1))
                    nc.vector.tensor_copy(
                        out=o_sb[:, oc * 512:(oc + 1) * 512], in_=o_ps)
                    if oc == 1:
                        nc.sync.dma_start(
                            out=out[t0:t0 + 128, 0:1024], in_=o_sb[:, 0:1024])
                nc.sync.dma_start(
                    out=out[t0:t0 + 128, 1024:2048], in_=o_sb[:, 1024:2048])

            for c in range(NCH):
                for hh in range(NHL):
                    if c == 0 and hh < 2:
                        continue
                    prev = attn_item(c, hh, prev)
                    if c > 0:
                        proj_item(4 * (c - 1) + hh)
            av_group(*prev)
            for tb in range(4 * (NCH - 1), TB):
                proj_item(tb)

    nc.compile()
    return nc


_NC_CACHE = {}


def get_nc():
    if "nc" not in _NC_CACHE:
        _NC_CACHE["nc"] = build()
    return _NC_CACHE["nc"]


def _tileT(a):
    """[T, X] f32 -> [128, TB, X] bf16 tiled for per-partition-contiguous DMA."""
    Tn, X = a.shape
    return np.ascontiguousarray(
        a.reshape(TB, 128, X).transpose(1, 0, 2).astype(BF))


def make_in_maps(x, ve, cos, sin, Wq, Wk, Wv, Wproj, Wgate):
    """Shard full inputs into 8 per-core input maps (2 batch x 4 head groups)."""
    x = np.asarray(x, np.float32)
    ve = np.asarray(ve, np.float32)
    cosn = np.ascontiguousarray(np.asarray(cos, np.float32)[0, :, 0, :])
    sinn = np.ascontiguousarray(np.asarray(sin, np.float32)[0, :, 0, :])
    Wq = np.asarray(Wq, np.float32)
    Wk = np.asarray(Wk, np.float32)
    Wv = np.asarray(Wv, np.float32)
    Wproj = np.asarray(Wproj, np.float32)
    Wgate = np.asarray(Wgate, np.float32)
    cos_t = _tileT(cosn)
    sin_t = _tileT(sinn)
    in_maps = []
    for core in range(8):
        b, g = divmod(core, 4)
        xb = x[b]
        xtc = np.ascontiguousarray(
            xb.reshape(TB, 128, CT, 128).transpose(3, 0, 2, 1).astype(BF))
        wq_t = np.ascontiguousarray(
            Wq[:, g * 512:(g + 1) * 512]
            .reshape(CT, 128, 512).transpose(1, 0, 2).astype(BF))
        wvx = np.zeros((C, 2 * HD + 1), np.float32)
        wvx[:, 0:HD] = Wk[:, g * 128:(g + 1) * 128]
        wvx[:, HD:2 * HD] = Wv[:, g * 128:(g + 1) * 128]
        wvx[0:GC, 2 * HD] = Wgate[:, g]
        wkv_t = np.ascontiguousarray(
            wvx.reshape(CT, 128, 2 * HD + 1).transpose(1, 0, 2).astype(BF))
        wp = Wproj[g * 512:(g + 1) * 512, :]
        wp_t = np.ascontiguousarray(
            wp.reshape(NHL, 128, OC, 512).transpose(1, 0, 2, 3).astype(BF))
        in_maps.append({
            "xt": xtc,
            "wq": wq_t,
            "wkv": wkv_t,
            "wproj": wp_t,
            "ve": _tileT(2.0 * ve[b][:, g * 128:(g + 1) * 128]),
            "cosn": cos_t,
            "sinn": sin_t,
        })
    return in_maps


def run_cores(in_maps, trace=False, **kw):
    nc = get_nc()
    return run_bass_kernel_spmd(nc, in_maps, core_ids=list(range(8)), trace=trace, **kw)


def kernel(**inputs):
    in_maps = make_in_maps(
        inputs["x"], inputs["ve"], inputs["cos"], inputs["sin"],
        inputs["Wq"], inputs["Wk"], inputs["Wv"], inputs["Wproj"], inputs["Wgate"])
    res = run_cores(in_maps)
    parts = [res.results[i]["out"] for i in range(8)]
    out = np.stack([
        parts[0] + parts[1] + parts[2] + parts[3],
        parts[4] + parts[5] + parts[6] + parts[7],
    ]).astype(np.float32)
    return out


# revision 20
# speedup vs baseline: 1.0137x; 1.0041x over previous
"""Trainium2 Bass kernel for a causal self-attention block (GQA + gated value
embedding + RoPE + QK-RMSNorm), sharded over 8 NeuronCores.

Sharding: 8 cores = 2 (batch) x 4 (kv-head groups).  Each core computes, for
its batch b and head-group g (4 q-heads + 1 kv-head):
    q/k/v projections, gated ve addition, RoPE, RMSNorm, causal attention,
    and the partial output projection  y_g @ Wproj[g*512:(g+1)*512, :].
The host sums the 4 per-group partials for each batch (the Wproj
contraction distributes over head groups).

All matmul operands are bf16 (PSUM accumulation stays fp32); the host
pre-transposes x so no on-device x transposes are needed; attention
S / AV / softmax-denominator run in one merged pipeline with the output
projection interleaved per chunk so the PE stream stays dense.

Self-contained: hardcodes shapes; accepts FULL inputs, returns FULL output.
"""

from contextlib import ExitStack

import ml_dtypes
import numpy as np

import concourse.bacc as bacc
import concourse.bass as bass
import concourse.mybir as mybir
import concourse.tile as tile
from concourse.bass_utils import run_bass_kernel_spmd
from concourse.masks import make_identity

F32 = mybir.dt.float32
BF16 = mybir.dt.bfloat16
I32 = mybir.dt.int32
AF = mybir.ActivationFunctionType
ALU = mybir.AluOpType
AX = mybir.AxisListType
BF = ml_dtypes.bfloat16

B, C, HD, NHL, GC = 2, 2048, 128, 4, 32  # NHL = local q heads per core
T = 2048
TB = T // 128   # token blocks
CT = C // 128   # contraction tiles for qkv
NCH = T // 512  # i-chunks for attention
OC = C // 512   # output chunks for proj
ISQ = 1.0 / float(np.sqrt(128.0))
S128 = float(np.sqrt(128.0))
QKS = 64.0  # host pre-scale on Wq/Wk so fp8 weights sit in the normal range
RSQRT_MAGIC = 0x5F3759DF
F8 = mybir.dt.float8e4
F8NP = ml_dtypes.float8_e4m3
DR = mybir.MatmulPerfMode.DoubleRow


def _bcast(ap_, idx, count):
    """Insert a step-0 (broadcast) dim at position idx of the AP dims."""
    lst = [list(p) for p in ap_.ap]
    lst.insert(idx, [0, count])
    return bass.AP(ap_.tensor, ap_.offset, lst)


def build():
    nc = bacc.Bacc("TRN2", target_bir_lowering=False, debug=False)
    xt = nc.dram_tensor("xt", [128, TB, CT, 128], BF16, kind="ExternalInput")
    wq = nc.dram_tensor("wq", [128, CT, 512], BF16, kind="ExternalInput")
    wkv = nc.dram_tensor("wkv", [128, CT, 2 * HD + 1], BF16, kind="ExternalInput")
    wproj = nc.dram_tensor("wproj", [128, NHL, OC, 512], BF16, kind="ExternalInput")
    ve = nc.dram_tensor("ve", [128, TB, HD], BF16, kind="ExternalInput")
    cosn = nc.dram_tensor("cosn", [128, TB, 64], BF16, kind="ExternalInput")
    sinn = nc.dram_tensor("sinn", [128, TB, 64], BF16, kind="ExternalInput")
    out = nc.dram_tensor("out", [T, C], F32, kind="ExternalOutput")

    with ExitStack() as stk:
        tc = stk.enter_context(tile.TileContext(nc))
        gpool = stk.enter_context(tc.tile_pool(name="gconst", bufs=1))
        warmsrc = gpool.tile([128, 128], BF16)
        nc.vector.memset(warmsrc, 1.0)
        ident = gpool.tile([128, 128], F32)
        make_identity(nc, ident)
        identb = gpool.tile([128, 128], BF16)
        nc.vector.tensor_copy(out=identb, in_=ident)
        ones_f = gpool.tile([128, 128], F32)
        nc.vector.memset(ones_f, 1.0)
        onesb = gpool.tile([128, 128], BF16)
        nc.vector.tensor_copy(out=onesb, in_=ones_f)
        # preload the Exp activation table while DMAs land
        tl_i = gpool.tile([128, 1], F32)
        nc.vector.memset(tl_i, 0.0)
        tl_o = gpool.tile([128, 1], F32)
        nc.scalar.activation(tl_o, tl_i, AF.Exp, scale=-1.0)

        persist = stk.enter_context(tc.tile_pool(name="persist", bufs=1))
        qkT = persist.tile([128, NHL + 1, T], BF16)  # [d, h, t]; h=4 is k
        vS = persist.tile([128, TB, HD], BF16)       # [t%128, t//128, d]
        yT = persist.tile([128, NHL, T], BF16)       # [d, h, t]

        wA = stk.enter_context(tc.tile_pool(name="wA", bufs=1))
        wq_sb = wA.tile([128, CT, 512], BF16)
        wkv_sb = wA.tile([128, CT, 2 * HD + 1], BF16)
        wp_sb = wA.tile([128, NHL, OC, 512], BF16)
        ve_sb = wA.tile([128, TB, HD], BF16)
        cos_sb = wA.tile([128, TB, 64], BF16)
        sin_sb = wA.tile([128, TB, 64], BF16)

        # DMA kickoff, interleaved across the two DGE queues so the first
        # qkv block's operands all land within a few us.
        xns = {}
        xpool = stk.enter_context(tc.tile_pool(name="xpool", bufs=5))
        for tb in range(4):
            xns[tb] = xpool.tile([128, CT, 128], BF16, tag="xn", name="xn")
        nc.gpsimd.dma_start(out=xns[0], in_=xt[:, 0])
        nc.sync.dma_start(out=wq_sb[:, 0:8, :], in_=wq[:, 0:8, :])
        nc.gpsimd.dma_start(out=wq_sb[:, 8:16, :], in_=wq[:, 8:16, :])
        nc.sync.dma_start(out=wkv_sb[:, 0:8, :], in_=wkv[:, 0:8, :])
        nc.gpsimd.dma_start(out=wkv_sb[:, 8:16, :], in_=wkv[:, 8:16, :])
        nc.sync.dma_start(out=xns[1], in_=xt[:, 1])
        nc.gpsimd.dma_start(out=xns[2], in_=xt[:, 2])
        nc.sync.dma_start(out=ve_sb, in_=ve[:, :, :])
        nc.gpsimd.dma_start(out=cos_sb, in_=cosn[:, :, :])
        nc.sync.dma_start(out=xns[3], in_=xt[:, 3])
        nc.gpsimd.dma_start(out=sin_sb, in_=sinn[:, :, :])
        nc.gpsimd.dma_start(out=wp_sb, in_=wproj[:, :, :, :])

        # PE warmup so HAM ramps toward full clock while the DMAs land.
        with tc.tile_pool(name="warm", bufs=2, space="PSUM") as warm:
            for _ in range(24):
                w_ps = warm.tile([128, 128], BF16, tag="wps", name="wps")
                nc.tensor.transpose(w_ps, warmsrc, warmsrc)

        # staged PSUM scopes: qkv-accumulation pools close before the
        # attention pools open; the transpose pool stays open into the
        # attention phase so the last block's transposes can be emitted
        # after the first attention items (PE never waits on the tail
        # of the phase-A DVE chain).
        psA1 = ExitStack()
        psq = psA1.enter_context(tc.tile_pool(name="psq", bufs=3, space="PSUM"))
        pskv = psA1.enter_context(tc.tile_pool(name="pskv", bufs=3, space="PSUM"))
        psA2 = ExitStack()
        pst = psA2.enter_context(
            tc.tile_pool(name="pst", bufs=2, space="PSUM", side="right"))

        sbA = stk.enter_context(tc.tile_pool(name="sbA", bufs=2))
        qkh = stk.enter_context(tc.tile_pool(name="qkh", bufs=3))

        def emit_transposes(pend):
            pqk, pt0 = pend
            for hh in range(NHL + 1):
                tq_ps = pst.tile([128, 128], BF16, tag="tps")
                nc.tensor.transpose(
                    tq_ps, pqk[:, hh * 128:(hh + 1) * 128], identb)
                nc.scalar.copy(out=qkT[:, hh, pt0:pt0 + 128], in_=tq_ps)

        # ---------------- phase A: qkv + rope + rmsnorm + transposes --------
        with nc.named_scope("phaseA"):
            pends = []  # (qkhat, t0) awaiting transpose into qkT, lag 2
            for tb in range(TB):
                t0 = tb * 128
                xn = xns.pop(tb)
                q_ps = psq.tile([128, NHL * HD], F32, tag="qps")
                kv_ps = pskv.tile([128, 2 * HD + 1], F32, tag="kvps")

                # qkv matmuls (k, v and the gate column fused in one rhs);
                # transposes of the block-before-last are interleaved
                # (2-block lag gives the DVE chain time to finish, so the
                # PE never waits on qkhat)
                pend = pends.pop(0) if len(pends) == 2 else None
                for ct in range(CT):
                    nc.tensor.matmul(
                        q_ps, lhsT=xn[:, ct, :], rhs=wq_sb[:, ct, :],
                        start=(ct == 0), stop=(ct == CT - 1))
                    nc.tensor.matmul(
                        kv_ps, lhsT=xn[:, ct, :], rhs=wkv_sb[:, ct, :],
                        start=(ct == 0), stop=(ct == CT - 1))
                    if pend is not None and ct in (2, 5, 8, 11, 14):
                        hh = (ct - 2) // 3
                        pqk, pt0 = pend
                        tq_ps = pst.tile([128, 128], BF16, tag="tps")
                        nc.tensor.transpose(
                            tq_ps, pqk[:, hh * 128:(hh + 1) * 128], identb)
                        nc.scalar.copy(
                            out=qkT[:, hh, pt0:pt0 + 128], in_=tq_ps)

                # prefetch: emitted after this block's matmuls so the ring
                # slot's previous reads are already ordered before the write
                if tb + 4 < TB:
                    xpf = xpool.tile([128, CT, 128], BF16, tag="xn", name="xn")
                    nc.sync.dma_start(out=xpf, in_=xt[:, tb + 4])
                    xns[tb + 4] = xpf

                # PSUM -> SBUF casts: q(4 heads)+k into one 5-head rope tile
                qkb = qkh.tile([128, (NHL + 1) * HD], BF16, tag="qkb")
                nc.scalar.copy(out=qkb[:, 0:512], in_=q_ps)
                nc.scalar.copy(out=qkb[:, 512:640], in_=kv_ps[:, 0:HD])
                vb = sbA.tile([128, HD], BF16, tag="vb")
                nc.scalar.copy(out=vb, in_=kv_ps[:, HD:2 * HD])

                # gate = sigmoid(z); z rides the kv matmul as weight col 256
                e_sb = sbA.tile([128, 1], F32, tag="esb")
                nc.scalar.activation(
                    e_sb, kv_ps[:, 2 * HD:2 * HD + 1], AF.Exp, scale=-1.0)
                nc.vector.tensor_scalar_add(e_sb, e_sb, 1.0)
                g_sb = sbA.tile([128, 1], F32, tag="gsb")
                nc.vector.reciprocal(g_sb, e_sb)
                # v = v_mm + sigmoid(z) * (2*ve)
                nc.vector.scalar_tensor_tensor(
                    out=vS[:, tb, :], in0=ve_sb[:, tb, :], scalar=g_sb,
                    in1=vb, op0=ALU.mult, op1=ALU.add)

                # ---- RoPE on q (4 heads) and k batched as 5 heads ----
                NH5 = NHL + 1
                cosB = _bcast(cos_sb[:, tb, :], 1, NH5)
                sinB = _bcast(sin_sb[:, tb, :], 1, NH5)
                qv = qkb.rearrange("p (h d) -> p h d", h=NH5)
                rh = sbA.tile([128, NH5 * HD], BF16, tag="rh")
                rhv = rh.rearrange("p (h d) -> p h d", h=NH5)
                tmp = sbA.tile([128, NH5, 64], BF16, tag="tmp")
                nc.vector.tensor_tensor(
                    out=rhv[:, :, 0:64], in0=qv[:, :, 0:64], in1=cosB,
                    op=ALU.mult)
                nc.vector.tensor_tensor(
                    out=tmp, in0=qv[:, :, 64:128], in1=sinB, op=ALU.mult)
                nc.vector.tensor_tensor(
                    out=rhv[:, :, 0:64], in0=rhv[:, :, 0:64], in1=tmp,
                    op=ALU.add)
                nc.vector.tensor_tensor(
                    out=rhv[:, :, 64:128], in0=qv[:, :, 64:128], in1=cosB,
                    op=ALU.mult)
                nc.vector.tensor_tensor(
                    out=tmp, in0=qv[:, :, 0:64], in1=sinB, op=ALU.mult)
                nc.vector.tensor_tensor(
                    out=rhv[:, :, 64:128], in0=rhv[:, :, 64:128], in1=tmp,
                    op=ALU.subtract)

                # ---- RMSNorm scales for 5 heads in one [128, 5] batch ----
                # rq = sqrt(128)*rsqrt(sum(q^2)) = rsqrt(mean(q^2)); the
                # sqrt(128) is folded into the Newton-step constants.
                sq2 = sbA.tile([128, NH5 * HD], BF16, tag="sq2")
                nc.vector.tensor_tensor(out=sq2, in0=rh, in1=rh, op=ALU.mult)
                red = sbA.tile([128, NH5], F32, tag="red")
                nc.vector.tensor_reduce(
                    out=red, in_=sq2.rearrange("p (h d) -> p h d", h=NH5),
                    axis=AX.X, op=ALU.add)
                rq = sbA.tile([128, NH5], F32, tag="rq")
                rqi = rq.bitcast(I32)
                nc.vector.tensor_scalar(
                    out=rqi, in0=red.bitcast(I32), scalar1=1, scalar2=None,
                    op0=ALU.logical_shift_right)
                nc.vector.tensor_scalar(
                    out=rqi, in0=rqi, scalar1=-1, scalar2=RSQRT_MAGIC,
                    op0=ALU.mult, op1=ALU.add)
                nt = sbA.tile([128, NH5], F32, tag="nt")
                nc.vector.tensor_tensor(out=nt, in0=rq, in1=rq, op=ALU.mult)
                nc.vector.tensor_tensor(out=nt, in0=nt, in1=red, op=ALU.mult)
                nc.vector.tensor_scalar(
                    out=nt, in0=nt, scalar1=-0.5 * S128, scalar2=1.5 * S128,
                    op0=ALU.mult, op1=ALU.add)
                nc.vector.tensor_tensor(out=rq, in0=rq, in1=nt, op=ALU.mult)

                qkhat = qkh.tile([128, NH5 * HD], BF16, tag="qkhat")
                for h5 in range(NH5):
                    nc.vector.tensor_scalar_mul(
                        qkhat[:, h5 * HD:(h5 + 1) * HD],
                        rhv[:, h5, :], rq[:, h5:h5 + 1])
                pends.append((qkhat, t0))

        psA1.close()  # free qkv PSUM banks for the attention pools

        # ---------------- phase B+C: attention + output projection ---------
        # c-outer / head-inner, software-pipelined S/exp one group ahead of
        # AV; softmax denominator via per-group DVE fold + one matmul; the
        # output projection for chunk c-1's four token blocks is interleaved
        # between heads of chunk c so the PE stream stays dense end-to-end.
        ptB = stk.enter_context(tc.tile_pool(name="ptB", bufs=6))
        smB = stk.enter_context(tc.tile_pool(name="smB", bufs=2))
        osb = stk.enter_context(tc.tile_pool(name="osb", bufs=2))
        with nc.named_scope("phaseBC"):
            psS = stk.enter_context(tc.tile_pool(name="psS", bufs=3, space="PSUM"))
            psy = stk.enter_context(tc.tile_pool(name="psy", bufs=2, space="PSUM"))
            psd = stk.enter_context(tc.tile_pool(name="psd", bufs=1, space="PSUM"))

            def s_group(meta, g):
                hh, c, i0 = meta["hh"], meta["c"], meta["i0"]
                pt = ptB.tile([128, 1024], BF16, tag="pt", name="pt")
                for s in range(2):
                    jb = 2 * g + s
                    io = max(0, 128 * jb - 512 * c)  # first causally-live col
                    sps = psS.tile([128, 512], F32, tag="sps", name="sps")
                    nc.tensor.matmul(
                        sps[:, io:512],
                        lhsT=qkT[:, NHL, jb * 128:(jb + 1) * 128],
                        rhs=qkT[:, hh, i0 + io:i0 + 512],
                        start=True, stop=True)
                    nc.scalar.activation(
                        pt[:, s * 512 + io:(s + 1) * 512],
                        sps[:, io:512], AF.Exp, scale=ISQ)
                for s in range(2):
                    jb = 2 * g + s
                    if jb >= 4 * c:  # diagonal block: zero j > i
                        io = 128 * (jb - 4 * c)
                        nc.gpsimd.affine_select(
                            out=pt[:, s * 512 + io:(s + 1) * 512],
                            in_=pt[:, s * 512 + io:(s + 1) * 512],
                            pattern=[[1, 512 - io]], compare_op=ALU.is_ge,
                            fill=0.0, base=0, channel_multiplier=-1)
                meta["pts"][g] = pt

            def av_group(meta, g, is_last):
                pt = meta["pts"].pop(g)
                yps = meta["yps"]
                c = meta["c"]
                for s in range(2):
                    jb = 2 * g + s
                    io = max(0, 128 * jb - 512 * c)
                    if jb == meta["first_jb"]:
                        io = 0  # start matmul must cover the full chunk
                    nc.tensor.matmul(
                        yps[:, io:512], lhsT=vS[:, jb, :],
                        rhs=pt[:, s * 512 + io:(s + 1) * 512],
                        start=(jb == meta["first_jb"]),
                        stop=(jb == meta["last_jb"]))
                # fold the two key blocks of this group on DVE, then one
                # per-group denominator matmul (halves the PE denominator)
                dps = meta["dps"]
                io0 = max(0, 128 * (2 * g) - 512 * c)
                io1 = max(0, 128 * (2 * g + 1) - 512 * c)
                acc = smB.tile([128, 512], BF16, tag="acc", name="acc")
                nc.vector.tensor_tensor(
                    out=acc[:, io1:512], in0=pt[:, io1:512],
                    in1=pt[:, 512 + io1:1024], op=ALU.add)
                if io1 > io0:
                    nc.vector.tensor_copy(
                        out=acc[:, io0:io1], in_=pt[:, io0:io1])
                nc.tensor.matmul(
                    dps[:, io0:512], lhsT=onesb, rhs=acc[:, io0:512],
                    start=(g == meta["order0"]),
                    stop=(g == meta["order_last"]))
                if is_last:
                    hh, i0 = meta["hh"], meta["i0"]
                    rcp = smB.tile([128, 512], F32, tag="rcp", name="rcp")
                    nc.vector.reciprocal_approx_fast(out=rcp, in_=dps)
                    nc.vector.tensor_tensor(
                        out=yT[:, hh, i0:i0 + 512], in0=yps, in1=rcp,
                        op=ALU.mult)

            def attn_item(c, hh, prev):
                yps = psy.tile([128, 512], F32, tag="yps", name="yps")
                dps = psd.tile([128, 512], F32, tag="dps", name="dps")
                ngrp = (4 * c + 4) // 2
                order = list(range(ngrp - 2, ngrp)) + list(range(ngrp - 2))
                meta = dict(hh=hh, c=c, i0=c * 512, yps=yps, dps=dps,
                            pts={}, first_jb=2 * order[0],
                            last_jb=2 * order[-1] + 1,
                            order0=order[0], order_last=order[-1])
                for idx, g in enumerate(order):
                    s_group(meta, g)
                    if prev is not None:
                        av_group(*prev)
                    prev = (meta, g, idx == len(order) - 1)
                return prev

            # first attention items run while the last qkv blocks' DVE
            # chains finish; their transposes are emitted in between.
            prev = attn_item(0, 0, None)
            emit_transposes(pends[0])
            prev = attn_item(0, 1, prev)
            emit_transposes(pends[1])
            psA2.close()  # free the transpose PSUM banks
            psC = stk.enter_context(tc.tile_pool(name="psC", bufs=2, space="PSUM"))

            def proj_item(tb):
                t0 = tb * 128
                o_sb = osb.tile([128, C], F32, tag="osb", name="osb")
                for oc in range(OC):
                    o_ps = psC.tile([128, 512], F32, tag="ops", name="ops")
                    for hh in range(NHL):
                        nc.tensor.matmul(
                            o_ps, lhsT=yT[:, hh, t0:t0 + 128],
                            rhs=wp_sb[:, hh, oc, :],
                            start=(hh == 0), stop=(hh == NHL - 1))
                    if oc % 2 == 0:
                        nc.scalar.copy(
                            out=o_sb[:, oc * 512:(oc + 1) * 512], in_=o_ps)
                    else:
                        nc.vector.tensor_copy(
                            out=o_sb[:, oc * 512:(oc + 1) * 512], in_=o_ps)
                    if oc == 1:
                        nc.sync.dma_start(
                            out=out[t0:t0 + 128, 0:1024], in_=o_sb[:, 0:1024])
                nc.sync.dma_start(
                    out=out[t0:t0 + 128, 1024:2048], in_=o_sb[:, 1024:2048])

            for c in range(NCH):
                for hh in range(NHL):
                    if c == 0 and hh < 2:
                        continue
                    prev = attn_item(c, hh, prev)
                    if c > 0:
                        proj_item(4 * (c - 1) + hh)
            av_group(*prev)
            for tb in range(4 * (NCH - 1), TB):
                proj_item(tb)

    nc.compile()
    return nc


_NC_CACHE = {}


def get_nc():
    if "nc" not in _NC_CACHE:
        _NC_CACHE["nc"] = build()
    return _NC_CACHE["nc"]


def _tileT(a):
    """[T, X] f32 -> [128, TB, X] bf16 tiled for per-partition-contiguous DMA."""
    Tn, X = a.shape
    return np.ascontiguousarray(
        a.reshape(TB, 128, X).transpose(1, 0, 2).astype(BF))


def make_in_maps(x, ve, cos, sin, Wq, Wk, Wv, Wproj, Wgate):
    """Shard full inputs into 8 per-core input maps (2 batch x 4 head groups)."""
    x = np.asarray(x, np.float32)
    ve = np.asarray(ve, np.float32)
    cosn = np.ascontiguousarray(np.asarray(cos, np.float32)[0, :, 0, :])
    sinn = np.ascontiguousarray(np.asarray(sin, np.float32)[0, :, 0, :])
    Wq = np.asarray(Wq, np.float32)
    Wk = np.asarray(Wk, np.float32)
    Wv = np.asarray(Wv, np.float32)
    Wproj = np.asarray(Wproj, np.float32)
    Wgate = np.asarray(Wgate, np.float32)
    cos_t = _tileT(cosn)
    sin_t = _tileT(sinn)
    in_maps = []
    for core in range(8):
        b, g = divmod(core, 4)
        xb = x[b]
        xtc = np.ascontiguousarray(
            xb.reshape(TB, 128, CT, 128).transpose(3, 0, 2, 1).astype(BF))
        wq_t = np.ascontiguousarray(
            Wq[:, g * 512:(g + 1) * 512]
            .reshape(CT, 128, 512).transpose(1, 0, 2).astype(BF))
        wvx = np.zeros((C, 2 * HD + 1), np.float32)
        wvx[:, 0:HD] = Wk[:, g * 128:(g + 1) * 128]
        wvx[:, HD:2 * HD] = Wv[:, g * 128:(g + 1) * 128]
        wvx[0:GC, 2 * HD] = Wgate[:, g]
        wkv_t = np.ascontiguousarray(
            wvx.reshape(CT, 128, 2 * HD + 1).transpose(1, 0, 2).astype(BF))
        wp = Wproj[g * 512:(g + 1) * 512, :]
        wp_t = np.ascontiguousarray(
            wp.reshape(NHL, 128, OC, 512).transpose(1, 0, 2, 3).astype(BF))
        in_maps.append({
            "xt": xtc,
            "wq": wq_t,
            "wkv": wkv_t,
            "wproj": wp_t,
            "ve": _tileT(2.0 * ve[b][:, g * 128:(g + 1) * 128]),
            "cosn": cos_t,
            "sinn": sin_t,
        })
    return in_maps


def run_cores(in_maps, trace=False, **kw):
    nc = get_nc()
    return run_bass_kernel_spmd(nc, in_maps, core_ids=list(range(8)), trace=trace, **kw)


def kernel(**inputs):
    in_maps = make_in_maps(
        inputs["x"], inputs["ve"], inputs["cos"], inputs["sin"],
        inputs["Wq"], inputs["Wk"], inputs["Wv"], inputs["Wproj"], inputs["Wgate"])
    res = run_cores(in_maps)
    parts = [res.results[i]["out"] for i in range(8)]
    out = np.stack([
        parts[0] + parts[1] + parts[2] + parts[3],
        parts[4] + parts[5] + parts[6] + parts[7],
    ]).astype(np.float32)
    return out
